# revision 1
# baseline (speedup 1.0000x reference)
"""CosformerAttention (causal linear attention) Trainium2 Bass kernel.

Full inputs in, full output out. Shards batch*heads over 8 NeuronCores:
device d handles sample n = d//4 and heads hA = 2*(d%4), hB = hA+1.
Per device: q/k/v projections for its 2 heads (bf16 matmuls), chunked
causal linear attention with prefix-summed inter-chunk states, and a
partial output projection over its 128 local features; the host sums
the 4 per-sample partials.

Self-contained: hardcodes L=1024, N=2, E=512, H=8 from the problem spec.
"""

import sys

if "/opt/trn_rl_repo" not in sys.path:
    sys.path.insert(0, "/opt/trn_rl_repo")

import numpy as np
import ml_dtypes

BF16NP = ml_dtypes.bfloat16

import concourse.bass as bass
import concourse.tile as tile
from concourse import mybir
import concourse.bass_utils as bass_utils
from concourse.vector_clock import ScopedClock

F32 = mybir.dt.float32
BF16 = mybir.dt.bfloat16
ALU = mybir.AluOpType
ACTF = mybir.ActivationFunctionType

L, N, E, H = 1024, 2, 512, 8
D = E // H          # 64 head dim
DD = 2 * D          # 128 cos/sin-doubled head dim
P = 128             # partitions / chunk size
NCHUNK = L // P     # 8
NCORES = 8
EPS = 1e-6


# ---------------------------------------------------------------------------
# This walrus build allows at most ONE semaphore wait per instruction.
# (a) Tile's tail drain carries the whole global clock: split it across
#     preceding SP nops.  (b) Skip the tail barriers + semaphore clearing --
#     the Bass preamble already dma_resets + sem_clears the entire kernel
#     semaphore range at program start, so end-of-kernel cleanup is
#     redundant and costs ~10us of EVSEM butterfly.
# ---------------------------------------------------------------------------
def _patched_drain_and_barrier(self, tick_clock, wait_clock):
    nc = self.nc
    drain_inst = nc.sync.drain()
    wait_clock.add_sem_waits(
        drain_inst.ins, ScopedClock({None: tick_clock.global_clock})
    )
    waits = list(drain_inst.ins.sync_info.on_wait or [])
    if len(waits) > 1:
        drain_inst.ins.sync_info.on_wait = [waits[0]]
        SI = type(drain_inst.ins.sync_info)
        for w in waits[1:]:
            nop = nc.sync.nop()
            si = nop.ins.sync_info
            if si is None:
                nop.ins.sync_info = SI(on_wait=[w], on_update=[])
            else:
                si.on_wait = [w]
    nc.all_engine_barrier()
    popped = nc._tile_sem_poison_stack.pop()
    assert popped is self._sem_poison


tile.TileContext._drain_and_barrier = _patched_drain_and_barrier


def _split_multi_waits(nc):
    """Move excess sem waits onto preceding same-engine NoOps (engines
    execute strictly in order, so this is equivalent)."""
    k = 0
    for f in nc.m.functions:
        for bb in f.blocks:
            insts = list(bb.instructions)
            out, changed = [], False
            for inst in insts:
                si = inst.sync_info
                waits = list(si.on_wait) if (si is not None and si.on_wait) else []
                if len(waits) > 1 and "Unassigned" not in str(inst.engine):
                    for w in waits[:-1]:
                        nop = mybir.InstNoOp(name=f"wsplit-{k}", ins=[], outs=[])
                        k += 1
                        nop.engine = inst.engine
                        nop.sync_info = type(si)(on_wait=[w], on_update=[])
                        out.append(nop)
                    si.on_wait = [waits[-1]]
                    changed = True
                out.append(inst)
            if changed:
                bb.instructions = out


def bcast(ap, dims):
    """Append broadcast (step 0) free dims to an AP."""
    return bass.AP(tensor=ap.tensor, offset=ap.offset,
                   ap=list(ap.ap) + [[0, d] for d in dims])


def build_program():
    nc = bass.Bass("TRN2", target_bir_lowering=False)

    # ---- DRAM I/O (packed to minimize DMA trigger count) -------------------
    # xT: (4*128, L) bf16 -- x transposed, e-major
    xT_d = nc.dram_tensor("xT", [E, L], BF16, kind="ExternalInput").ap()
    # w_all: (512, 768) bf16 = [wq_dup (256) | wk_dup (256) | w_vk (256)]
    w_d = nc.dram_tensor("w_all", [E, 768], BF16, kind="ExternalInput").ap()
    # wb16: (128, 640) bf16 = [outwT (512) | ident (128)]
    wb_d = nc.dram_tensor("wb16", [P, 640], BF16, kind="ExternalInput").ap()
    # cf32: (128, 1172) f32 =
    #   [sc_full 0:1024 | mask 1024:1152 | s_col 1152:1160 | c_col 1160:1168 |
    #    qb 1168:1170 | kb 1170:1172]
    cf_d = nc.dram_tensor("cf32", [P, 1172], F32, kind="ExternalInput").ap()
    # row1: (1, 384) bf16 = [vkb (256) | ones (128)]
    row1_d = nc.dram_tensor("row1", [1, 384], BF16, kind="ExternalInput").ap()
    out_d = nc.dram_tensor("out", [L, E], F32, kind="ExternalOutput").ap()

    with tile.TileContext(nc) as tc:
        persist = tc.alloc_tile_pool(name="persist", bufs=1)
        work = tc.alloc_tile_pool(name="work", bufs=3)
        small = tc.alloc_tile_pool(name="small", bufs=4)
        ps_big = tc.alloc_tile_pool(name="ps_big", bufs=2, space="PSUM")
        ps_misc = tc.alloc_tile_pool(name="ps_misc", bufs=1, space="PSUM")
        ps_po = tc.alloc_tile_pool(name="ps_po", bufs=3, space="PSUM")

        # ---- batched input loads ------------------------------------------
        xT = persist.tile([P, 4, L], BF16, tag="xT", name="xT")
        nc.sync.dma_start(out=xT[:], in_=xT_d.rearrange("(e p) l -> p e l", p=P))
        w_all = persist.tile([P, 4, 768], BF16, tag="w_all", name="w_all")
        nc.sync.dma_start(out=w_all[:], in_=w_d.rearrange("(e p) f -> p e f", p=P))
        wb16 = persist.tile([P, 640], BF16, tag="wb16", name="wb16")
        nc.sync.dma_start(out=wb16[:], in_=wb_d)
        cf32 = persist.tile([P, 1172], F32, tag="cf32", name="cf32")
        nc.sync.dma_start(out=cf32[:], in_=cf_d)
        row1 = persist.tile([1, 384], BF16, tag="row1", name="row1")
        nc.sync.dma_start(out=row1[:], in_=row1_d)

        def wq(e):
            return w_all[:, e, 0:256]

        def wk(e):
            return w_all[:, e, 256:512]

        def wvk(e):
            return w_all[:, e, 512:768]

        outw = wb16[:, 0:512]
        ident = wb16[:, 512:640]
        sc = cf32[:, 0:1024]
        mask = cf32[:, 1024:1152]
        scol = cf32[:, 1152:1160]
        ccol = cf32[:, 1160:1168]
        vkb = row1[:, 0:256]
        ones_row = row1[:, 256:384]

        # persistent activations
        q_f = [persist.tile([P, L], BF16, tag=f"qf{h}", name=f"qf{h}") for h in range(2)]
        k_f = [persist.tile([P, L], BF16, tag=f"kf{h}", name=f"kf{h}") for h in range(2)]
        # k_t: [ch, head, sc, d] sequence-layout scaled k
        k_t = persist.tile([P, NCHUNK, 2, 2, D], BF16, tag="kt", name="kt")
        # v_t: [ch, head, d+1] with ones column
        v_t = persist.tile([P, NCHUNK, 2, D + 1], BF16, tag="vt", name="vt")
        attn = persist.tile([P, NCHUNK, P], BF16, tag="attn", name="attn")
        Sc_sb = persist.tile([P, NCHUNK, 2, D + 1], BF16, tag="scsb", name="scsb")
        Spfx = persist.tile([P, NCHUNK, 2, D + 1], BF16, tag="spfx", name="spfx")
        aT = persist.tile([P, NCHUNK, P], BF16, tag="aT", name="aT")

        # ---- stage B: feature-layout q_/k_ ((2d, L), scaled by sin/cos) ----
        for si in range(4):
            wsel = wq if si < 2 else wk
            bcol = 1168 + si  # qbA, qbB, kbA, kbB (dup'd bias columns)
            h = si % 2
            dst = q_f[h] if si < 2 else k_f[h]
            for tch in range(2):
                ps = ps_big.tile([P, 512], F32, tag="big")
                for e in range(4):
                    nc.tensor.matmul(
                        ps[:],
                        wsel(e)[:, h * P:(h + 1) * P],
                        xT[:, e, tch * 512:(tch + 1) * 512],
                        start=(e == 0),
                        stop=(e == 3),
                    )
                tmp = work.tile([P, 512], F32, tag="brelu")
                nc.scalar.activation(
                    tmp[:], ps[:], ACTF.Relu, bias=cf32[:, bcol:bcol + 1], scale=1.0
                )
                nc.vector.tensor_mul(
                    dst[:, tch * 512:(tch + 1) * 512],
                    tmp[:],
                    sc[:, tch * 512:(tch + 1) * 512],
                )

        # ---- stage C: sequence-layout v (ones col) and scaled k ------------
        # psum cols: 0:64 vA, 64:128 vB, 128:192 kA, 192:256 kB
        for ch in range(NCHUNK):
            ps = ps_big.tile([P, 256], F32, tag="big")
            nc.tensor.matmul(ps[:], ones_row[:], vkb[:], start=True, stop=False)
            for e in range(4):
                nc.tensor.matmul(ps[:], xT[:, e, ch * P:(ch + 1) * P], wvk(e),
                                 start=False, stop=(e == 3))
            # v: one strided copy for both heads + ones col
            nc.vector.tensor_copy(
                v_t[:, ch, :, 0:D],
                ps[:, 0:128].rearrange("p (h d) -> p h d", h=2),
            )
            nc.vector.memset(v_t[:, ch, :, D:D + 1], 1.0)
            # k_t: relu+scale on ACT (scale AP is per-partition; s,c > 0 so
            # relu(x)*s == relu(x*s))
            kc = ps[:, 128:256].rearrange("p (h d) -> p h d", h=2)
            nc.scalar.activation(k_t[:, ch, :, 0, :], kc, ACTF.Relu,
                                 scale=scol[:, ch:ch + 1])
            nc.scalar.activation(k_t[:, ch, :, 1, :], kc, ACTF.Relu,
                                 scale=ccol[:, ch:ch + 1])

        # ---- stage D1: per-chunk local states + prefix sum -----------------
        for ch in range(NCHUNK):
            psc = ps_po.tile([P, 2, D + 1], F32, tag="po130")
            for h in range(2):
                nc.tensor.matmul(psc[:, h, :], k_t[:, ch, h, :, :],
                                 v_t[:, ch, h, :], start=True, stop=True)
            nc.scalar.activation(Sc_sb[:, ch, :, :], psc[:], ACTF.Copy)
        nc.vector.tensor_copy(Spfx[:, 1], Sc_sb[:, 0])
        for ch in range(2, NCHUNK):
            nc.vector.tensor_add(Spfx[:, ch], Spfx[:, ch - 1], Sc_sb[:, ch - 1])

        # ---- stage D2: per-chunk attention ---------------------------------
        for ch in range(NCHUNK):
            cs = slice(ch * P, (ch + 1) * P)
            po = ps_po.tile([P, 2, D + 1], F32, tag="po130")
            for h in range(2):
                pss = ps_misc.tile([P, P], F32, tag="sq", bufs=2)
                nc.tensor.matmul(pss[:], k_f[h][:, cs], q_f[h][:, cs],
                                 start=True, stop=True)
                ms = work.tile([P, P], BF16, tag="ms")
                nc.vector.tensor_mul(ms[:], pss[:], mask[:])
                nc.tensor.matmul(po[:, h, :], ms[:], v_t[:, ch, h, :],
                                 start=True, stop=(ch == 0))
                if ch > 0:
                    nc.tensor.matmul(po[:, h, :], q_f[h][:, cs],
                                     Spfx[:, ch, h, :], start=False, stop=True)
            den = small.tile([P, 2], F32, tag="den")
            nc.vector.tensor_scalar(den[:], po[:, :, D], scalar1=EPS,
                                    scalar2=None, op0=ALU.max)
            rec = small.tile([P, 2], F32, tag="rec")
            nc.vector.reciprocal(rec[:], den[:])
            nc.vector.tensor_mul(
                attn[:, ch, :].rearrange("p (h d) -> p h d", h=2),
                po[:, :, 0:D],
                bcast(rec[:, :], [D]),
            )

        # ---- stage E: transpose attn (batched 4/bank) + output proj --------
        for g in range(2):
            tp = ps_misc.tile([P, 4, P], BF16, tag="tp", bufs=1)
            for i in range(4):
                nc.tensor.transpose(tp[:, i, :], attn[:, g * 4 + i, :], ident)
            nc.vector.tensor_copy(aT[:, g * 4:(g + 1) * 4, :], tp[:])
            for i in range(4):
                ch = g * 4 + i
                pso = ps_big.tile([P, E], F32, tag="big")
                nc.tensor.matmul(pso[:], aT[:, ch, :], outw, start=True, stop=True)
                osb = work.tile([P, E], F32, tag="osb")
                nc.scalar.activation(osb[:], pso[:], ACTF.Copy)
                nc.sync.dma_start(out=out_d[ch * P:(ch + 1) * P, :], in_=osb[:])

        for p in (ps_po, ps_misc, ps_big, small, work, persist):
            p.release()

    _split_multi_waits(nc)
    return nc


_PROG = {}


def _get_program():
    if "nc" not in _PROG:
        _PROG["nc"] = build_program()
    return _PROG["nc"]


def _prep_core_inputs(dev, query, q_w, q_b, k_w, k_b, v_w, v_b, out_w):
    n = dev // 4
    hA = 2 * (dev % 4)
    a, b = hA * D, (hA + 1) * D

    def dup(w, lo):
        wt = w[lo:lo + D, :].T  # (E, 64)
        return np.concatenate([wt, wt], axis=1)  # (E, 128)

    xT = np.ascontiguousarray(query[:, n, :].T.astype(np.float32))
    wq_f = np.concatenate([dup(q_w, a), dup(q_w, b)], axis=1)     # (E, 256)
    wk_f = np.concatenate([dup(k_w, a), dup(k_w, b)], axis=1)     # (E, 256)
    w_vk = np.concatenate(
        [v_w[a:a + D, :].T, v_w[b:b + D, :].T,
         k_w[a:a + D, :].T, k_w[b:b + D, :].T], axis=1)           # (E, 256)
    w_all = np.concatenate([wq_f, wk_f, w_vk], axis=1)            # (E, 768)
    outwT = np.concatenate([out_w[:, a:a + D].T, out_w[:, b:b + D].T], axis=0)
    wb16 = np.concatenate([outwT, np.eye(P, dtype=np.float32)], axis=1)

    idx = np.arange(1, L + 1, dtype=np.float64) * (np.pi / 2) / L
    s = np.sin(idx).astype(np.float32)
    c = np.cos(idx).astype(np.float32)
    sc_full = np.concatenate(
        [np.broadcast_to(s, (D, L)), np.broadcast_to(c, (D, L))], axis=0
    ).astype(np.float32)
    s_col = np.ascontiguousarray(s.reshape(NCHUNK, P).T)
    c_col = np.ascontiguousarray(c.reshape(NCHUNK, P).T)
    pi = np.arange(P)
    mask = (pi[:, None] <= pi[None, :]).astype(np.float32)
    qb_f = np.stack(
        [np.concatenate([q_b[a:a + D]] * 2), np.concatenate([q_b[b:b + D]] * 2)],
        axis=1).astype(np.float32)
    kb_f = np.stack(
        [np.concatenate([k_b[a:a + D]] * 2), np.concatenate([k_b[b:b + D]] * 2)],
        axis=1).astype(np.float32)
    cf32 = np.concatenate([sc_full, mask, s_col, c_col, qb_f, kb_f], axis=1)
    vkb = np.concatenate(
        [v_b[a:a + D], v_b[b:b + D], k_b[a:a + D], k_b[b:b + D]])
    row1 = np.concatenate(
        [vkb.astype(np.float32), np.ones(P, np.float32)]).reshape(1, 384)

    return {
        "xT": xT.astype(BF16NP),
        "w_all": np.ascontiguousarray(w_all).astype(BF16NP),
        "wb16": np.ascontiguousarray(wb16).astype(BF16NP),
        "cf32": np.ascontiguousarray(cf32.astype(np.float32)),
        "row1": row1.astype(BF16NP),
    }


def run(inputs, trace=False, trace_kwargs=None):
    nc = _get_program()
    in_maps = [
        _prep_core_inputs(
            d, inputs["query"], inputs["q_w"], inputs["q_b"], inputs["k_w"],
            inputs["k_b"], inputs["v_w"], inputs["v_b"], inputs["out_w"])
        for d in range(NCORES)
    ]
    res = bass_utils.run_bass_kernel_spmd(
        nc, in_maps, list(range(NCORES)), trace=trace,
        **(trace_kwargs or {}),
    )
    parts = [res.results[i]["out"] for i in range(NCORES)]
    out0 = parts[0] + parts[1] + parts[2] + parts[3]
    out1 = parts[4] + parts[5] + parts[6] + parts[7]
    out = np.stack([out0, out1], axis=1) + inputs["out_b"][None, None, :]
    return out.astype(np.float32), res


def kernel(**inputs) -> np.ndarray:
    out, _ = run(inputs, trace=False)
    return out



# revision 8
# speedup vs baseline: 1.0626x; 1.0626x over previous
"""CosformerAttention (causal linear attention) Trainium2 Bass kernel.

Full inputs in, full output out. Shards batch*heads over 8 NeuronCores:
device d handles sample n = d//4 and heads hA = 2*(d%4), hB = hA+1.
Per device: q/k/v projections for its 2 heads (bf16 matmuls), chunked
causal linear attention with prefix-summed inter-chunk states, and a
partial output projection over its 128 local features; the host sums
the 4 per-sample partials (bf16 partials, f32 host accumulation).

v2 layout: undup'd projections (both heads in one 128-wide matmul
block), sin/cos applied by DVE muls with partition-crossing writes,
split early-start input DMAs, PE warmup matmuls during the load, and
bf16 output partials.

Self-contained: hardcodes L=1024, N=2, E=512, H=8 from the problem spec.
"""

import sys

if "/opt/trn_rl_repo" not in sys.path:
    sys.path.insert(0, "/opt/trn_rl_repo")

import numpy as np
import ml_dtypes

BF16NP = ml_dtypes.bfloat16

import concourse.bass as bass
import concourse.tile as tile
from concourse import mybir
import concourse.bass_utils as bass_utils
from concourse.vector_clock import ScopedClock

F32 = mybir.dt.float32
BF16 = mybir.dt.bfloat16
ALU = mybir.AluOpType
ACTF = mybir.ActivationFunctionType

L, N, E, H = 1024, 2, 512, 8
D = E // H          # 64 head dim
P = 128             # partitions / chunk size
NCHUNK = L // P     # 8
NCORES = 8
EPS = 1e-6
NWARM = 6           # PE warmup matmuls (p-state ramp) during input DMA


# ---------------------------------------------------------------------------
# This walrus build allows at most ONE semaphore wait per instruction.
# (a) Tile's tail drain carries the whole global clock: split it across
#     preceding SP nops.  (b) Skip the tail barriers + semaphore clearing --
#     the Bass preamble already dma_resets + sem_clears the entire kernel
#     semaphore range at program start, so end-of-kernel cleanup is
#     redundant and costs ~10us of EVSEM butterfly.
# ---------------------------------------------------------------------------
def _patched_drain_and_barrier(self, tick_clock, wait_clock):
    nc = self.nc
    drain_inst = nc.sync.drain()
    wait_clock.add_sem_waits(
        drain_inst.ins, ScopedClock({None: tick_clock.global_clock})
    )
    waits = list(drain_inst.ins.sync_info.on_wait or [])
    if len(waits) > 1:
        drain_inst.ins.sync_info.on_wait = [waits[0]]
        SI = type(drain_inst.ins.sync_info)
        for w in waits[1:]:
            nop = nc.sync.nop()
            si = nop.ins.sync_info
            if si is None:
                nop.ins.sync_info = SI(on_wait=[w], on_update=[])
            else:
                si.on_wait = [w]
    nc.all_engine_barrier()
    popped = nc._tile_sem_poison_stack.pop()
    assert popped is self._sem_poison


tile.TileContext._drain_and_barrier = _patched_drain_and_barrier


def _split_multi_waits(nc):
    """Move excess sem waits onto preceding same-engine NoOps (engines
    execute strictly in order, so this is equivalent)."""
    k = 0
    for f in nc.m.functions:
        for bb in f.blocks:
            insts = list(bb.instructions)
            out, changed = [], False
            for inst in insts:
                si = inst.sync_info
                waits = list(si.on_wait) if (si is not None and si.on_wait) else []
                if len(waits) > 1 and "Unassigned" not in str(inst.engine):
                    for w in waits[:-1]:
                        nop = mybir.InstNoOp(name=f"wsplit-{k}", ins=[], outs=[])
                        k += 1
                        nop.engine = inst.engine
                        nop.sync_info = type(si)(on_wait=[w], on_update=[])
                        out.append(nop)
                    si.on_wait = [waits[-1]]
                    changed = True
                out.append(inst)
            if changed:
                bb.instructions = out


def bcast(ap, dims):
    """Append broadcast (step 0) free dims to an AP."""
    return bass.AP(tensor=ap.tensor, offset=ap.offset,
                   ap=list(ap.ap) + [[0, d] for d in dims])


# misc16 column map (bf16): scS 0:1024 | scC 1024:2048 | mask 2048:2176 |
#   outw 2176:2688 | ident 2688:2816
MISC16_COLS = 2816
# misc32 column map (f32): scol 0:8 | ccol 8:16 | qb 16 | kb 17 | vkb row 18.. (unused)
MISC32_COLS = 20


def build_program(has_kb=False):
    nc = bass.Bass("TRN2", target_bir_lowering=False)

    # ---- DRAM I/O (host pre-packed partition-major) ------------------------
    # xT: [p, tch, e, l] packed as [128, 4096] bf16
    xT_d = nc.dram_tensor("xT", [P, 2 * 4 * 512], BF16, kind="ExternalInput").ap()
    # w: [p, e, col] cols = [q 0:128 | k 128:256 | vk 256:512] as [128, 2048]
    w_d = nc.dram_tensor("w", [P, 4 * 512], BF16, kind="ExternalInput").ap()
    misc16_d = nc.dram_tensor("misc16", [P, MISC16_COLS], BF16,
                              kind="ExternalInput").ap()
    misc32_d = nc.dram_tensor("misc32", [P, MISC32_COLS], F32,
                              kind="ExternalInput").ap()
    if has_kb:
        # [1, 384]: [0]*128 (v cols) + kb dup (128) | ones (128)
        kbrow_d = nc.dram_tensor("kbrow", [1, 384], BF16, kind="ExternalInput").ap()
    out_d = nc.dram_tensor("out", [L, E], BF16, kind="ExternalOutput").ap()

    with tile.TileContext(nc) as tc:
        persist = tc.alloc_tile_pool(name="persist", bufs=1)
        work = tc.alloc_tile_pool(name="work", bufs=3)
        small = tc.alloc_tile_pool(name="small", bufs=4)
        # PSUM budget (8 banks): ps_a 2 + ps_st 2 + ps_d2 2, then ps_st is
        # released and ps_eo (tp 1 + out 2) reuses its banks.
        ps_a = tc.alloc_tile_pool(name="ps_a", bufs=2, space="PSUM")
        ps_d2 = tc.alloc_tile_pool(name="ps_d2", bufs=2, space="PSUM")
        ps_st = tc.alloc_tile_pool(name="ps_st", bufs=2, space="PSUM")

        # ---- persistent tiles ---------------------------------------------
        xT = persist.tile([P, 2, 4, 512], BF16, tag="xT", name="xT")
        w_all = persist.tile([P, 4, 512], BF16, tag="w", name="w")
        misc16 = persist.tile([P, MISC16_COLS], BF16, tag="m16", name="m16")
        misc32 = persist.tile([P, MISC32_COLS], F32, tag="m32", name="m32")
        warm = persist.tile([P, 512], BF16, tag="warm", name="warm")
        q_f = [persist.tile([P, L], BF16, tag=f"qf{h}", name=f"qf{h}")
               for h in range(2)]
        k_f = [persist.tile([P, L], BF16, tag=f"kf{h}", name=f"kf{h}")
               for h in range(2)]
        # k_t: [seq, ch, head, sc, d]; v_t: [seq, ch, head, d+1] (ones col)
        k_t = persist.tile([P, NCHUNK, 2, 2, D], BF16, tag="kt", name="kt")
        v_t = persist.tile([P, NCHUNK, 2, D + 1], BF16, tag="vt", name="vt")
        Spfx = persist.tile([P, NCHUNK, 2, D + 1], BF16, tag="spfx", name="spfx")
        attn = persist.tile([P, NCHUNK, P], BF16, tag="attn", name="attn")
        aT = persist.tile([P, NCHUNK, P], BF16, tag="aT", name="aT")
        osb = persist.tile([P, NCHUNK, E], BF16, tag="osb", name="osb")
        if has_kb:
            kbrow = persist.tile([1, 384], BF16, tag="kbrow", name="kbrow")

        scS = misc16[:, 0:1024]
        scC = misc16[:, 1024:2048]
        mask = misc16[:, 2048:2176]
        outw = misc16[:, 2176:2688]
        ident = misc16[:, 2688:2816]
        scol = misc32[:, 0:8]
        ccol = misc32[:, 8:16]
        qb = misc32[:, 16:17]
        kb = misc32[:, 17:18]

        # ---- PE warmup (p-state ramp) + input DMAs ------------------------
        nc.gpsimd.memset(warm[:], 0.0)
        # ones column of v_t (all chunks/heads at once, strided)
        nc.gpsimd.memset(v_t[:, :, :, D:D + 1], 1.0)

        # SP triggers: wqk, xT half 0, xT half 1, wvk (in need order)
        nc.sync.dma_start(out=w_all[:, :, 0:256],
                          in_=w_d.rearrange("p (e c) -> p e c", e=4)[:, :, 0:256])
        nc.sync.dma_start(out=xT[:, 0], in_=xT_d[:, 0:2048].rearrange(
            "p (e l) -> p e l", e=4))
        nc.sync.dma_start(out=xT[:, 1], in_=xT_d[:, 2048:4096].rearrange(
            "p (e l) -> p e l", e=4))
        nc.sync.dma_start(out=w_all[:, :, 256:512],
                          in_=w_d.rearrange("p (e c) -> p e c", e=4)[:, :, 256:512])
        # ACT triggers: misc16, misc32 (+ kbrow)
        nc.scalar.dma_start(out=misc16[:], in_=misc16_d)
        nc.scalar.dma_start(out=misc32[:], in_=misc32_d)
        if has_kb:
            nc.scalar.dma_start(out=kbrow[:], in_=kbrow_d)

        for i in range(NWARM):
            pw = ps_a.tile([P, 512], F32, tag="big")
            nc.tensor.matmul(pw[:], warm[:, 0:128], warm[:], start=True, stop=True)

        # ---- stage B: feature-layout q/k, both heads undup'd --------------
        # psum [128 = qA(64)|qB(64), 512]; relu+bias -> bf16 tmp;
        # 4 DVE muls spread to the dup'd scaled layout per head.
        for si in range(2):            # 0 = q, 1 = k
            wcol = slice(si * 128, (si + 1) * 128)
            bias = qb if si == 0 else kb
            dst = q_f if si == 0 else k_f
            for tch in range(2):
                cs = slice(tch * 512, (tch + 1) * 512)
                ps = ps_a.tile([P, 512], F32, tag="big")
                for e in range(4):
                    nc.tensor.matmul(ps[:], w_all[:, e, wcol],
                                     xT[:, tch, e, :], start=(e == 0),
                                     stop=(e == 3))
                tmp = work.tile([P, 512], BF16, tag="brelu")
                nc.scalar.activation(tmp[:], ps[:], ACTF.Relu, bias=bias, scale=1.0)
                nc.vector.tensor_mul(dst[0][0:64, cs], tmp[0:64, :], scS[0:64, cs])
                nc.vector.tensor_mul(dst[0][64:128, cs], tmp[0:64, :], scC[0:64, cs])
                nc.vector.tensor_mul(dst[1][0:64, cs], tmp[64:128, :],
                                     scS[64:128, cs])
                nc.vector.tensor_mul(dst[1][64:128, cs], tmp[64:128, :],
                                     scC[64:128, cs])

        # ---- stage C: sequence-layout v (+ones) and scaled k --------------
        # psum [128(seq), 2ch, 256]: cols 0:64 vA | 64:128 vB | 128:192 kA |
        # 192:256 kB per sub-chunk
        for g in range(NCHUNK // 2):
            ps = ps_a.tile([P, 2, 256], F32, tag="big")
            for sub in range(2):
                ch = 2 * g + sub
                tch, lo = ch // 4, (ch % 4) * P
                if has_kb:
                    nc.tensor.matmul(ps[:, sub], kbrow[:, 256:384],
                                     kbrow[:, 0:256], start=True, stop=False)
                for e in range(4):
                    nc.tensor.matmul(ps[:, sub], xT[:, tch, e, lo:lo + P],
                                     w_all[:, e, 256:512],
                                     start=(e == 0 and not has_kb),
                                     stop=(e == 3))
            # v: one strided copy for both sub-chunks & heads
            nc.vector.tensor_copy(
                v_t[:, 2 * g:2 * g + 2, :, 0:D],
                ps[:, :, 0:128].rearrange("p s (h d) -> p s h d", h=2),
            )
            for sub in range(2):
                ch = 2 * g + sub
                kc = ps[:, sub, 128:256].rearrange("p (h d) -> p h d", h=2)
                nc.scalar.activation(k_t[:, ch, :, 0, :], kc, ACTF.Relu,
                                     scale=scol[:, ch:ch + 1])
                nc.scalar.activation(k_t[:, ch, :, 1, :], kc, ACTF.Relu,
                                     scale=ccol[:, ch:ch + 1])

        # ---- stage D1: per-chunk local states + prefix sum (from psum) ----
        pscs = []
        for ch in range(NCHUNK - 1):   # last chunk's state never needed
            psc = ps_st.tile([P, 2, D + 1], F32, tag="st")
            for h in range(2):
                nc.tensor.matmul(psc[:, h, :], k_t[:, ch, h, :, :],
                                 v_t[:, ch, h, :], start=True, stop=True)
            pscs.append(psc)
        nc.vector.tensor_copy(Spfx[:, 1], pscs[0][:])
        for ch in range(2, NCHUNK):
            nc.vector.tensor_add(Spfx[:, ch], Spfx[:, ch - 1], pscs[ch - 1][:])
        ps_st.release()
        # tp (1) + out (2) banks reuse the released ps_st space
        ps_eo = tc.alloc_tile_pool(name="ps_eo", bufs=1, space="PSUM")

        # ---- stage D2: per-chunk attention --------------------------------
        # One bank per chunk: pssA 0:128 | pssB 128:256 | po 256:386
        for ch in range(NCHUNK):
            cs = slice(ch * P, (ch + 1) * P)
            d2 = ps_d2.tile([P, 386], F32, tag="d2")
            po = d2[:, 256:386].rearrange("p (h v) -> p h v", h=2)
            for h in range(2):
                pss = d2[:, h * P:(h + 1) * P]
                nc.tensor.matmul(pss, k_f[h][:, cs], q_f[h][:, cs],
                                 start=True, stop=True)
                ms = work.tile([P, P], BF16, tag="ms")
                nc.vector.tensor_mul(ms[:], pss, mask[:])
                nc.tensor.matmul(po[:, h, :], ms[:], v_t[:, ch, h, :],
                                 start=True, stop=(ch == 0))
                if ch > 0:
                    nc.tensor.matmul(po[:, h, :], q_f[h][:, cs],
                                     Spfx[:, ch, h, :], start=False, stop=True)
            den = small.tile([P, 2], F32, tag="den")
            nc.vector.tensor_scalar(den[:], po[:, :, D], scalar1=EPS,
                                    scalar2=None, op0=ALU.max)
            rec = small.tile([P, 2], F32, tag="rec")
            nc.vector.reciprocal(rec[:], den[:])
            nc.vector.tensor_mul(
                attn[:, ch, :].rearrange("p (h d) -> p h d", h=2),
                po[:, :, 0:D],
                bcast(rec[:, :], [D]),
            )

        # ---- stage E: transpose attn (2/bank) + output proj + store -------
        for g in range(NCHUNK // 2):
            tp = ps_eo.tile([P, 2, P], BF16, tag="tp", bufs=1)
            for i in range(2):
                nc.tensor.transpose(tp[:, i, :], attn[:, 2 * g + i, :], ident)
            nc.vector.tensor_copy(aT[:, 2 * g:2 * g + 2, :], tp[:])
            for i in range(2):
                ch = 2 * g + i
                pso = ps_eo.tile([P, E], F32, tag="out", bufs=2)
                nc.tensor.matmul(pso[:], aT[:, ch, :], outw, start=True,
                                 stop=True)
                nc.scalar.activation(osb[:, ch, :], pso[:], ACTF.Copy)
            nc.sync.dma_start(
                out=out_d.rearrange("(c p) e -> p c e", p=P)[:, 2 * g:2 * g + 2, :],
                in_=osb[:, 2 * g:2 * g + 2, :])

        for p in (ps_eo, ps_d2, ps_a, small, work, persist):
            p.release()

    _split_multi_waits(nc)
    return nc


_PROG = {}


def _get_program(has_kb):
    if has_kb not in _PROG:
        _PROG[has_kb] = build_program(has_kb)
    return _PROG[has_kb]


def _prep_core_inputs(dev, query, q_w, q_b, k_w, k_b, v_w, v_b, out_w):
    n = dev // 4
    hA = 2 * (dev % 4)
    aA, aB = hA * D, (hA + 1) * D

    x = np.asarray(query[:, n, :], np.float32)          # (L, E)
    xT = x.reshape(2, 512, 4, P).transpose(3, 0, 2, 1)  # (p, tch, e, l)
    xT = np.ascontiguousarray(xT.reshape(P, 4096))

    def blk(w):
        # (p, e, 128): cols = head A feats 0:64, head B feats 64:128
        b = np.concatenate([w[aA:aA + D, :], w[aB:aB + D, :]], 0).T  # (512,128)
        return b.reshape(4, P, P).transpose(1, 0, 2)

    wq = blk(np.asarray(q_w, np.float32))
    wk = blk(np.asarray(k_w, np.float32))
    # vk cols: vA 0:64 | vB 64:128 | kA 128:192 | kB 192:256
    vk = np.concatenate([v_w[aA:aA + D, :], v_w[aB:aB + D, :],
                         k_w[aA:aA + D, :], k_w[aB:aB + D, :]], 0).T  # (512,256)
    wvk = vk.reshape(4, P, 256).transpose(1, 0, 2)
    w_pack = np.ascontiguousarray(
        np.concatenate([wq, wk, wvk], axis=2).reshape(P, 2048))

    idx = np.arange(1, L + 1, dtype=np.float64) * (np.pi / 2) / L
    s = np.sin(idx).astype(np.float32)
    c = np.cos(idx).astype(np.float32)
    scS = np.broadcast_to(s, (P, L))
    scC = np.broadcast_to(c, (P, L))
    pi = np.arange(P)
    mask = (pi[:, None] <= pi[None, :]).astype(np.float32)
    outw = np.concatenate([out_w[:, aA:aA + D].T, out_w[:, aB:aB + D].T], 0)
    misc16 = np.concatenate(
        [scS, scC, mask, outw, np.eye(P, dtype=np.float32)], axis=1)

    s_col = np.ascontiguousarray(s.reshape(NCHUNK, P).T)
    c_col = np.ascontiguousarray(c.reshape(NCHUNK, P).T)
    qb_col = np.concatenate([q_b[aA:aA + D], q_b[aB:aB + D]])[:, None]
    kb_col = np.concatenate([k_b[aA:aA + D], k_b[aB:aB + D]])[:, None]
    pad = np.zeros((P, MISC32_COLS - 18), np.float32)
    misc32 = np.concatenate([s_col, c_col, qb_col, kb_col, pad],
                            axis=1).astype(np.float32)

    ins = {
        "xT": xT.astype(BF16NP),
        "w": w_pack.astype(BF16NP),
        "misc16": np.ascontiguousarray(misc16).astype(BF16NP),
        "misc32": np.ascontiguousarray(misc32),
    }
    if np.any(np.asarray(k_b) != 0):
        kbrow = np.concatenate(
            [np.zeros(128, np.float32),
             k_b[aA:aA + D], k_b[aB:aB + D]]).reshape(1, 256)
        ins["kbrow"] = kbrow.astype(BF16NP)
    return ins


def run(inputs, trace=False, trace_kwargs=None):
    has_kb = bool(np.any(np.asarray(inputs["k_b"]) != 0))
    nc = _get_program(has_kb)
    in_maps = [
        _prep_core_inputs(
            d, inputs["query"], inputs["q_w"], inputs["q_b"], inputs["k_w"],
            inputs["k_b"], inputs["v_w"], inputs["v_b"], inputs["out_w"])
        for d in range(NCORES)
    ]
    res = bass_utils.run_bass_kernel_spmd(
        nc, in_maps, list(range(NCORES)), trace=trace,
        **(trace_kwargs or {}),
    )
    parts = [res.results[i]["out"].astype(np.float32) for i in range(NCORES)]
    out0 = parts[0] + parts[1] + parts[2] + parts[3]
    out1 = parts[4] + parts[5] + parts[6] + parts[7]
    # v_b passes through attention verbatim: its out-proj image folds into
    # the output bias exactly.
    bias = (np.asarray(inputs["out_b"], np.float32)
            + np.asarray(inputs["out_w"], np.float32)
            @ np.asarray(inputs["v_b"], np.float32))
    out = np.stack([out0, out1], axis=1) + bias[None, None, :]
    return out.astype(np.float32), res


def kernel(**inputs) -> np.ndarray:
    out, _ = run(inputs, trace=False)
    return out


# revision 11
# speedup vs baseline: 1.1666x; 1.0979x over previous
"""CosformerAttention (causal linear attention) Trainium2 Bass kernel.

Full inputs in, full output out. Shards batch*heads over 8 NeuronCores:
device d handles sample n = d//4 and heads hA = 2*(d%4), hB = hA+1.
Per device: q/k/v projections for its 2 heads (bf16 matmuls), chunked
causal linear attention with prefix-summed inter-chunk states, and a
partial output projection over its 128 local features; the host sums
the 4 per-sample partials (bf16 partials, f32 host accumulation).

v2.1: undup'd projections (both heads per 128-wide matmul block) with
sin/cos applied by a few wide DVE muls (partition-crossing writes);
k_t produced by PE-transposing k_f (no ACT relu pass); v-only seq
projections; batched mask muls; split early-start input DMAs; PE
warmup matmuls and an ACT-table preload during the load; gpsimd
partition-broadcast for the sin/cos rows; bf16 output partials.

Self-contained: hardcodes L=1024, N=2, E=512, H=8 from the problem spec.
"""

import sys

if "/opt/trn_rl_repo" not in sys.path:
    sys.path.insert(0, "/opt/trn_rl_repo")

import numpy as np
import ml_dtypes

BF16NP = ml_dtypes.bfloat16

import concourse.bass as bass
import concourse.tile as tile
from concourse import mybir
import concourse.bass_utils as bass_utils
from concourse.vector_clock import ScopedClock

F32 = mybir.dt.float32
BF16 = mybir.dt.bfloat16
ALU = mybir.AluOpType
ACTF = mybir.ActivationFunctionType

L, N, E, H = 1024, 2, 512, 8
D = E // H          # 64 head dim
P = 128             # partitions / chunk size
NCHUNK = L // P     # 8
NCORES = 8
EPS = 1e-6
NWARM = 10          # PE warmup matmuls (p-state ramp) during input DMA


# ---------------------------------------------------------------------------
# This walrus build allows at most ONE semaphore wait per instruction.
# (a) Tile's tail drain carries the whole global clock: split it across
#     preceding SP nops.  (b) Skip the tail barriers + semaphore clearing --
#     the Bass preamble already dma_resets + sem_clears the entire kernel
#     semaphore range at program start, so end-of-kernel cleanup is
#     redundant and costs ~10us of EVSEM butterfly.
# ---------------------------------------------------------------------------
def _patched_drain_and_barrier(self, tick_clock, wait_clock):
    nc = self.nc
    drain_inst = nc.sync.drain()
    wait_clock.add_sem_waits(
        drain_inst.ins, ScopedClock({None: tick_clock.global_clock})
    )
    waits = list(drain_inst.ins.sync_info.on_wait or [])
    if len(waits) > 1:
        drain_inst.ins.sync_info.on_wait = [waits[0]]
        SI = type(drain_inst.ins.sync_info)
        for w in waits[1:]:
            nop = nc.sync.nop()
            si = nop.ins.sync_info
            if si is None:
                nop.ins.sync_info = SI(on_wait=[w], on_update=[])
            else:
                si.on_wait = [w]
    nc.all_engine_barrier()
    popped = nc._tile_sem_poison_stack.pop()
    assert popped is self._sem_poison


tile.TileContext._drain_and_barrier = _patched_drain_and_barrier


def _split_multi_waits(nc):
    """Move excess sem waits onto preceding same-engine NoOps (engines
    execute strictly in order, so this is equivalent)."""
    k = 0
    for f in nc.m.functions:
        for bb in f.blocks:
            insts = list(bb.instructions)
            out, changed = [], False
            for inst in insts:
                si = inst.sync_info
                waits = list(si.on_wait) if (si is not None and si.on_wait) else []
                if len(waits) > 1 and "Unassigned" not in str(inst.engine):
                    for w in waits[:-1]:
                        nop = mybir.InstNoOp(name=f"wsplit-{k}", ins=[], outs=[])
                        k += 1
                        nop.engine = inst.engine
                        nop.sync_info = type(si)(on_wait=[w], on_update=[])
                        out.append(nop)
                    si.on_wait = [waits[-1]]
                    changed = True
                out.append(inst)
            if changed:
                bb.instructions = out


def bcast(ap, dims):
    """Append broadcast (step 0) free dims to an AP."""
    return bass.AP(tensor=ap.tensor, offset=ap.offset,
                   ap=list(ap.ap) + [[0, d] for d in dims])


def bcast_mid(ap, n):
    """Insert a broadcast (step 0) dim after the partition dim of a 2D AP."""
    a = list(ap.ap)
    return bass.AP(tensor=ap.tensor, offset=ap.offset,
                   ap=[a[0], [0, n]] + a[1:])


# late16 column map (bf16): ident 0:128 | mask 128:256 | outw 256:768
LATE16_COLS = 768
# misc32 column map (f32): qb 0 | kb 1 | pad
MISC32_COLS = 4


def build_program():
    nc = bass.Bass("TRN2", target_bir_lowering=False)

    # ---- DRAM I/O (host pre-packed partition-major) ------------------------
    # xT: [p, tch, e, l] packed as [128, 4096] bf16
    xT_d = nc.dram_tensor("xT", [P, 2 * 4 * 512], BF16, kind="ExternalInput").ap()
    # w: [p, e, col] cols = [q 0:128 | k 128:256 | v 256:384] as [128, 1536]
    w_d = nc.dram_tensor("w", [P, 4 * 384], BF16, kind="ExternalInput").ap()
    # sc16: [sin bcast 0:1024 | cos bcast 1024:2048] on all partitions
    sc16_d = nc.dram_tensor("sc16", [P, 2 * L], BF16, kind="ExternalInput").ap()
    late16_d = nc.dram_tensor("late16", [P, LATE16_COLS], BF16,
                              kind="ExternalInput").ap()
    misc32_d = nc.dram_tensor("misc32", [P, MISC32_COLS], F32,
                              kind="ExternalInput").ap()
    out_d = nc.dram_tensor("out", [L, E], BF16, kind="ExternalOutput").ap()

    with tile.TileContext(nc) as tc:
        persist = tc.alloc_tile_pool(name="persist", bufs=1)
        work = tc.alloc_tile_pool(name="work", bufs=3)
        small = tc.alloc_tile_pool(name="small", bufs=4)
        # PSUM budget (8 banks): ps_a 2 + ps_d2 2 + ps_ktp 2 + ps_st 2,
        # then st+ktp are released and ps_eo (tp 1 + out 2) reuses them.
        ps_a = tc.alloc_tile_pool(name="ps_a", bufs=2, space="PSUM")
        ps_d2 = tc.alloc_tile_pool(name="ps_d2", bufs=2, space="PSUM")
        ps_ktp = tc.alloc_tile_pool(name="ps_ktp", bufs=2, space="PSUM")
        ps_st = tc.alloc_tile_pool(name="ps_st", bufs=2, space="PSUM")

        # ---- persistent tiles ---------------------------------------------
        xT = persist.tile([P, 2, 4, 512], BF16, tag="xT", name="xT")
        w_all = persist.tile([P, 4, 384], BF16, tag="w", name="w")
        sc16 = persist.tile([P, 2 * L], BF16, tag="sc16", name="sc16")
        late16 = persist.tile([P, LATE16_COLS], BF16, tag="l16", name="l16")
        misc32 = persist.tile([P, MISC32_COLS], F32, tag="m32", name="m32")
        warm = persist.tile([P, 512], BF16, tag="warm", name="warm")
        # tmpqk: relu'd undup'd features [qA|qB ; kA|kB] x [tch halves]
        tmpq = persist.tile([P, L], BF16, tag="tmpq", name="tmpq")
        tmpk = persist.tile([P, L], BF16, tag="tmpk", name="tmpk")
        q_f = [persist.tile([P, L], BF16, tag=f"qf{h}", name=f"qf{h}")
               for h in range(2)]
        k_f = [persist.tile([P, L], BF16, tag=f"kf{h}", name=f"kf{h}")
               for h in range(2)]
        # k_t: [seq, ch, head, sc, d]; v_t: [seq, ch, head, d+1] (ones col)
        k_t = persist.tile([P, NCHUNK, 2, 2, D], BF16, tag="kt", name="kt")
        v_t = persist.tile([P, NCHUNK, 2, D + 1], BF16, tag="vt", name="vt")
        Spfx = persist.tile([P, NCHUNK, 2, D + 1], BF16, tag="spfx", name="spfx")
        attn = persist.tile([P, NCHUNK, P], BF16, tag="attn", name="attn")
        aT = persist.tile([P, NCHUNK, P], BF16, tag="aT", name="aT")
        osb = persist.tile([P, NCHUNK, E], BF16, tag="osb", name="osb")

        ident = late16[:, 0:128]
        mask = late16[:, 128:256]
        outw = late16[:, 256:768]
        qb = misc32[:, 0:1]
        kb = misc32[:, 1:2]

        # ---- warmup, table preload, input DMAs ----------------------------
        nc.gpsimd.memset(warm[:], 0.0)
        nc.gpsimd.memset(v_t[:, :, :, D:D + 1], 1.0)

        # SP triggers (need order): w_qk, xT half 0, xT half 1, w_v
        nc.sync.dma_start(out=w_all[:, :, 0:256],
                          in_=w_d.rearrange("p (e c) -> p e c", e=4)[:, :, 0:256])
        nc.sync.dma_start(out=xT[:, 0], in_=xT_d[:, 0:2048].rearrange(
            "p (e l) -> p e l", e=4))
        nc.sync.dma_start(out=xT[:, 1], in_=xT_d[:, 2048:4096].rearrange(
            "p (e l) -> p e l", e=4))
        nc.sync.dma_start(out=w_all[:, :, 256:384],
                          in_=w_d.rearrange("p (e c) -> p e c", e=4)[:, :, 256:384])
        nc.sync.dma_start(out=sc16[:], in_=sc16_d)
        # ACT triggers: misc32 (bias, needed by first relu), late16
        nc.scalar.dma_start(out=misc32[:], in_=misc32_d)
        nc.scalar.dma_start(out=late16[:], in_=late16_d)

        # ACT PWP table preload (Relu) while DMAs run
        dum = work.tile([P, 8], BF16, tag="dum")
        nc.scalar.activation(dum[:], warm[:, 0:8], ACTF.Relu, scale=1.0)

        scS = sc16[:, 0:L]
        scC = sc16[:, L:2 * L]

        for i in range(NWARM):
            pw = ps_a.tile([P, 512], F32, tag="big")
            nc.tensor.matmul(pw[:], warm[:, 0:128], warm[:], start=True, stop=True)

        # ---- stage B: feature-layout q/k, both heads undup'd --------------
        for si in range(2):            # 0 = q, 1 = k
            wcol = slice(si * 128, (si + 1) * 128)
            bias = qb if si == 0 else kb
            tmp = tmpq if si == 0 else tmpk
            for tch in range(2):
                cs = slice(tch * 512, (tch + 1) * 512)
                ps = ps_a.tile([P, 512], F32, tag="big")
                for e in range(4):
                    nc.tensor.matmul(ps[:], w_all[:, e, wcol],
                                     xT[:, tch, e, :], start=(e == 0),
                                     stop=(e == 3))
                nc.scalar.activation(tmp[:, cs], ps[:], ACTF.Relu, bias=bias,
                                     scale=1.0)
            dst = q_f if si == 0 else k_f
            nc.vector.tensor_mul(dst[0][0:64, :], tmp[0:64, :], scS[0:64, :])
            nc.vector.tensor_mul(dst[0][64:128, :], tmp[0:64, :], scC[0:64, :])
            nc.vector.tensor_mul(dst[1][0:64, :], tmp[64:128, :], scS[64:128, :])
            nc.vector.tensor_mul(dst[1][64:128, :], tmp[64:128, :],
                                 scC[64:128, :])

        # ---- stage C: sequence-layout v (4 chunks per bank) ---------------
        for g in range(2):
            ps = ps_a.tile([P, 512], F32, tag="big")
            pv = ps.rearrange("p (s c) -> p s c", s=4)
            for sub in range(4):
                ch = 4 * g + sub
                tch, lo = ch // 4, (ch % 4) * P
                for e in range(4):
                    nc.tensor.matmul(pv[:, sub, :], xT[:, tch, e, lo:lo + P],
                                     w_all[:, e, 256:384],
                                     start=(e == 0), stop=(e == 3))
            nc.vector.tensor_copy(
                v_t[:, 4 * g:4 * g + 4, :, 0:D],
                pv.rearrange("p s (h d) -> p s h d", h=2),
            )

        # ---- k_t: PE-transpose k_f chunks (4 chunks x 2 heads per bank) ---
        for g in range(2):
            ktp = ps_ktp.tile([P, 4, 2, P], BF16, tag="ktp")
            for sub in range(4):
                ch = 4 * g + sub
                cs = slice(ch * P, (ch + 1) * P)
                for h in range(2):
                    nc.tensor.transpose(ktp[:, sub, h, :], k_f[h][:, cs], ident)
            nc.vector.tensor_copy(
                k_t[:, 4 * g:4 * g + 4].rearrange("p c h s d -> p c h (s d)"),
                ktp[:],
            )

        # ---- stage D1: per-chunk local states + prefix sum (from psum) ----
        pscs = []
        for ch in range(NCHUNK - 1):   # last chunk's state never needed
            psc = ps_st.tile([P, 2, D + 1], F32, tag="st")
            for h in range(2):
                nc.tensor.matmul(psc[:, h, :], k_t[:, ch, h, :, :],
                                 v_t[:, ch, h, :], start=True, stop=True)
            pscs.append(psc)
        nc.vector.tensor_copy(Spfx[:, 1], pscs[0][:])
        for ch in range(2, NCHUNK):
            nc.vector.tensor_add(Spfx[:, ch], Spfx[:, ch - 1], pscs[ch - 1][:])
        ps_st.release()
        ps_ktp.release()
        # tp (1) + out (2) banks reuse the released space
        ps_eo = tc.alloc_tile_pool(name="ps_eo", bufs=1, space="PSUM")

        # ---- stage D2: per-chunk attention --------------------------------
        # One bank per chunk: pssA 0:128 | pssB 128:256 | po 256:386
        for ch in range(NCHUNK):
            cs = slice(ch * P, (ch + 1) * P)
            d2 = ps_d2.tile([P, 386], F32, tag="d2")
            po = d2[:, 256:386].rearrange("p (h v) -> p h v", h=2)
            for h in range(2):
                nc.tensor.matmul(d2[:, h * P:(h + 1) * P], k_f[h][:, cs],
                                 q_f[h][:, cs], start=True, stop=True)
            ms = work.tile([P, 2, P], BF16, tag="ms")
            nc.vector.tensor_mul(
                ms[:], d2[:, 0:256].rearrange("p (h q) -> p h q", h=2),
                bcast_mid(mask, 2))
            for h in range(2):
                nc.tensor.matmul(po[:, h, :], ms[:, h, :], v_t[:, ch, h, :],
                                 start=True, stop=(ch == 0))
                if ch > 0:
                    nc.tensor.matmul(po[:, h, :], q_f[h][:, cs],
                                     Spfx[:, ch, h, :], start=False, stop=True)
            den = small.tile([P, 2], F32, tag="den")
            nc.vector.tensor_scalar(den[:], po[:, :, D], scalar1=EPS,
                                    scalar2=None, op0=ALU.max)
            rec = small.tile([P, 2], F32, tag="rec")
            nc.vector.reciprocal(rec[:], den[:])
            nc.vector.tensor_mul(
                attn[:, ch, :].rearrange("p (h d) -> p h d", h=2),
                po[:, :, 0:D],
                bcast(rec[:, :], [D]),
            )

        # ---- stage E: transpose attn (4/bank) + output proj + store -------
        for g in range(2):
            tp = ps_eo.tile([P, 4, P], BF16, tag="tp", bufs=1)
            for i in range(4):
                nc.tensor.transpose(tp[:, i, :], attn[:, 4 * g + i, :], ident)
            nc.vector.tensor_copy(aT[:, 4 * g:4 * g + 4, :], tp[:])
            for i in range(4):
                ch = 4 * g + i
                pso = ps_eo.tile([P, E], F32, tag="out", bufs=2)
                nc.tensor.matmul(pso[:], aT[:, ch, :], outw, start=True,
                                 stop=True)
                nc.scalar.activation(osb[:, ch, :], pso[:], ACTF.Copy)
                if ch % 2 == 1:
                    nc.sync.dma_start(
                        out=out_d.rearrange("(c p) e -> p c e", p=P)[:, ch - 1:ch + 1, :],
                        in_=osb[:, ch - 1:ch + 1, :])

        for p in (ps_eo, ps_d2, ps_a, small, work, persist):
            p.release()

    _split_multi_waits(nc)
    return nc


_PROG = {}


def _get_program():
    if "nc" not in _PROG:
        _PROG["nc"] = build_program()
    return _PROG["nc"]


def _prep_core_inputs(dev, query, q_w, q_b, k_w, k_b, v_w, v_b, out_w):
    n = dev // 4
    hA = 2 * (dev % 4)
    aA, aB = hA * D, (hA + 1) * D

    x = np.asarray(query[:, n, :], np.float32)          # (L, E)
    xT = x.reshape(2, 512, 4, P).transpose(3, 0, 2, 1)  # (p, tch, e, l)
    xT = np.ascontiguousarray(xT.reshape(P, 4096))

    def blk(w):
        # (p, e, 128): cols = head A feats 0:64, head B feats 64:128
        b = np.concatenate([w[aA:aA + D, :], w[aB:aB + D, :]], 0).T  # (512,128)
        return b.reshape(4, P, P).transpose(1, 0, 2)

    wq = blk(np.asarray(q_w, np.float32))
    wk = blk(np.asarray(k_w, np.float32))
    wv = blk(np.asarray(v_w, np.float32))
    w_pack = np.ascontiguousarray(
        np.concatenate([wq, wk, wv], axis=2).reshape(P, 4 * 384))

    idx = np.arange(1, L + 1, dtype=np.float64) * (np.pi / 2) / L
    s = np.sin(idx).astype(np.float32)
    c = np.cos(idx).astype(np.float32)
    sc16 = np.broadcast_to(np.concatenate([s, c]), (P, 2 * L))

    pi = np.arange(P)
    mask = (pi[:, None] <= pi[None, :]).astype(np.float32)
    outw = np.concatenate([out_w[:, aA:aA + D].T, out_w[:, aB:aB + D].T], 0)
    late16 = np.concatenate([np.eye(P, dtype=np.float32), mask, outw], axis=1)

    qb_col = np.concatenate([q_b[aA:aA + D], q_b[aB:aB + D]])[:, None]
    kb_col = np.concatenate([k_b[aA:aA + D], k_b[aB:aB + D]])[:, None]
    pad = np.zeros((P, MISC32_COLS - 2), np.float32)
    misc32 = np.concatenate([qb_col, kb_col, pad], axis=1).astype(np.float32)

    return {
        "xT": xT.astype(BF16NP),
        "w": w_pack.astype(BF16NP),
        "sc16": np.ascontiguousarray(sc16).astype(BF16NP),
        "late16": np.ascontiguousarray(late16).astype(BF16NP),
        "misc32": np.ascontiguousarray(misc32),
    }


def run(inputs, trace=False, trace_kwargs=None):
    nc = _get_program()
    in_maps = [
        _prep_core_inputs(
            d, inputs["query"], inputs["q_w"], inputs["q_b"], inputs["k_w"],
            inputs["k_b"], inputs["v_w"], inputs["v_b"], inputs["out_w"])
        for d in range(NCORES)
    ]
    res = bass_utils.run_bass_kernel_spmd(
        nc, in_maps, list(range(NCORES)), trace=trace,
        **(trace_kwargs or {}),
    )
    parts = [res.results[i]["out"].astype(np.float32) for i in range(NCORES)]
    out0 = parts[0] + parts[1] + parts[2] + parts[3]
    out1 = parts[4] + parts[5] + parts[6] + parts[7]
    # v_b passes through attention verbatim: its out-proj image folds into
    # the output bias exactly.
    bias = (np.asarray(inputs["out_b"], np.float32)
            + np.asarray(inputs["out_w"], np.float32)
            @ np.asarray(inputs["v_b"], np.float32))
    out = np.stack([out0, out1], axis=1) + bias[None, None, :]
    return out.astype(np.float32), res


def kernel(**inputs) -> np.ndarray:
    out, _ = run(inputs, trace=False)
    return out
